# revision 28
# baseline (speedup 1.0000x reference)
"""GATv2 message-passing kernel for 8 Trainium2 NeuronCores (Bass/Tile), v3.

Strategy: shard by RECEIVER RANGE. Host sorts edges by (receiver tile,
sender parity); core c owns output nodes [c*6272, (c+1)*6272) and the edges
pointing at them. Each receiver-tile of 128 nodes gets a static edge-slot
capacity C = G*128: the first G/2 groups hold even-sender edges, the last
G/2 odd-sender edges (padded slots masked). Per core:
  phase 1:  project all nodes -> DRAM table [NPAD, 64] f32 (replicated)
  phase 1b: project own node range -> SBUF-resident rtab [128, 49, 64]
  phase 2:  per chunk (= one parity half of a receiver tile, ng groups):
    - dense edge-feature load (transposed, ones-row folds We_bias)
    - ONE batched dma_gather (InstDMAGatherAnt, mlp GPSIMD library) pulls
      all ng*128 send rows: idx = sender>>1 (int16), gathering the 256B
      even/odd column half of the pairs-view table [NPAD/2, 128]
    - one-hot R [slot, j] built on DVE (is_equal vs iota, bf16);
      PE-transposed to R_T, ACT-copied to SBUF in 4-group batches
    - PE: x_psum = eft^T @ We_ext + R_T^T @ rtab_tile; DVE adds the
      gathered send rows in ONE batched add
    - DVE Mish (manual), logits, ACT Exp -> w, mask; payload w*send in one
      batched 4D mul
    - PE: psn += R^T @ payload accumulates the segment softmax sum
      [128 nodes, 72] (no scatters, no collectives)
  divide, store output shard directly. Host concatenates the 8 shards.
"""
import sys

sys.path.insert(0, "/opt/trn_rl_repo")
import numpy as np
import ml_dtypes
import concourse.bass as bass
import concourse.mybir as mybir
import concourse.tile as tile
import concourse.tile as tile_mod
from concourse import library_config
from concourse.masks import make_identity
from concourse.vector_clock import ScopedClock

# --- walrus build workarounds (same as baseline kernel.py) ---
try:
    from jax.interpreters import mlir as _mlir
    from concourse.bass2jax import (
        _bass_exec_p as _bep,
        _bass_exec_neuron_lowering as _benl,
        _partition_id_p as _pip,
        _partition_id_lowering as _pil,
    )

    _mlir.register_lowering(_bep, _benl, platform="axon")
    _mlir.register_lowering(_pip, _pil, platform="axon")
except Exception:  # pragma: no cover
    pass

_N_CARRIERS = 24


def _patched_drain_and_barrier(self, tick_clock, wait_clock):
    nc = self.nc
    nops = [nc.sync.nop(nofuse=True) for _ in range(_N_CARRIERS)]
    drain_inst = nc.sync.drain()
    wait_clock.add_sem_waits(
        drain_inst.ins, ScopedClock({None: tick_clock.global_clock}))
    waits = list(drain_inst.ins.sync_info.on_wait or [])
    if len(waits) > 1:
        assert len(waits) - 1 <= _N_CARRIERS
        drain_inst.ins.sync_info.on_wait = waits[:1]
        for nop, w in zip(nops, waits[1:]):
            si = nop.ins.sync_info
            if si is None:
                nop.ins.sync_info = mybir.SyncInfo(on_wait=[w], on_update=[])
            else:
                si.on_wait = [w]
    nc.all_engine_barrier()
    assert self.sems is not None
    popped = nc._tile_sem_poison_stack.pop()
    assert popped is self._sem_poison
    nc.clear_and_free_semaphores(list(self.sems.allocated().values()))
    nc.all_engine_barrier()


tile_mod.TileContext._drain_and_barrier = _patched_drain_and_barrier


def _split_excess_waits(nc, max_waits=1):
    for bbname, body in nc.bb_map.items():
        bb = body.bb
        insts = list(bb.instructions)
        out = []
        changed = False
        for ins in insts:
            si = ins.sync_info
            waits = list(si.on_wait) if si and si.on_wait else []
            if len(waits) > max_waits:
                keep = waits[:max_waits - 1] + [waits[-1]]
                extra = waits[max_waits - 1:-1]
                for w in extra:
                    nop = mybir.InstNoOp(
                        name=nc.get_next_instruction_name(), ins=[], outs=[])
                    nop.engine = ins.engine
                    nop.sync_info = mybir.SyncInfo(on_wait=[w], on_update=[])
                    nc.register_instruction(nop, overwrite=True)
                    out.append(nop)
                ins.sync_info.on_wait = keep
                changed = True
            out.append(ins)
        if changed:
            bb.instructions = out


F32 = mybir.dt.float32
BF16 = mybir.dt.bfloat16
I16 = mybir.dt.int16
F8 = mybir.dt.float8e4

N_NODES = 50000
N_EDGES = 1200000
IN_DIM = 128
EDGE_DIM = 64
EMBED = 64
HEADS = 8
PAY = EMBED + HEADS  # 72

N_CORES = 8
NPC = 6272            # nodes per core = 49 tiles of 128
NTILE = NPC // 128    # 49
NPAD = NPC * N_CORES  # 50176
MR = 4                # node-tiles per phase-1 iteration
MR2 = 7               # node-tiles per phase-1b iteration
NQ = 4                # SWDGE queues for gather rotation


def _ap3(ap, mid_n):
    """[128, D] AP -> [128, mid_n(step0), D] broadcast view."""
    return bass.AP(ap.tensor, ap.offset, [ap.ap[0], [0, mid_n]] + list(ap.ap[1:]))


def _inner_b(ap, n):
    """Append a step-0 innermost free dim of size n (broadcast view)."""
    return bass.AP(ap.tensor, ap.offset, list(ap.ap) + [[0, n]])


def build_nc(G, reps=1, batch_payload=True, xdt=None):
    """G = edge-slot groups (of 128) per receiver tile (even); the first G/2
    groups of each tile hold even-sender edges, the rest odd-sender."""
    assert G % 2 == 0
    if xdt is None:
        xdt = BF16
    C = G * 128
    SUB = G // 2  # groups per chunk; chunk 0 = even senders, chunk 1 = odd
    assert SUB >= MR2, "phase-1 PSUM tiles alias the phase-2 'px' ring"
    nc = bass.Bass(num_swdge_queues=NQ)

    nfT = nc.declare_dram_parameter("nfT", [IN_DIM, NPAD], BF16, isOutput=False)
    rnfT = nc.declare_dram_parameter("rnfT", [IN_DIM, NPC], BF16, isOutput=False)
    eftT = nc.declare_dram_parameter("eftT", [EDGE_DIM + 1, NTILE * C], BF16,
                                     isOutput=False)
    s16_e = nc.declare_dram_parameter("s16", [128, NTILE * G * 8], I16,
                                      isOutput=False)
    Rh_e = nc.declare_dram_parameter("Rh", [128, NTILE * G * 128], F8,
                                     isOutput=False)
    RhT_e = nc.declare_dram_parameter("RhT", [128, NTILE * G * 128], F8,
                                      isOutput=False)
    W_e = nc.declare_dram_parameter("W", [IN_DIM, EMBED], BF16, isOutput=False)
    Wb_e = nc.declare_dram_parameter("Wb", [128, EMBED], F32, isOutput=False)
    We_e = nc.declare_dram_parameter("We_ext", [EDGE_DIM + 1, EMBED], BF16,
                                     isOutput=False)
    a_e = nc.declare_dram_parameter("a64", [128, EMBED], BF16, isOutput=False)
    out_e = nc.declare_dram_parameter("out_shard", [NPC, EMBED], F32, isOutput=True)

    table = nc.dram_tensor("ntable", [NPAD, EMBED], F32)
    tap = table[:]
    # pairs views: row k of [NPAD/2, 128] = nodes 2k | 2k+1; gather idx = s>>1
    ev_view = bass.AP(tap.tensor, 0, [[2 * EMBED, NPAD // 2], [1, EMBED]])
    od_view = bass.AP(tap.tensor, EMBED, [[2 * EMBED, NPAD // 2], [1, EMBED]])

    # Load the GPSIMD library holding InstDMAGatherAnt in the main bb so it
    # executes strictly before every tile-scheduled gather (the tile
    # scheduler may reorder dependency-free instructions within a block).
    nc.gpsimd.load_library(library_config.mlp)
    ngA = (G // 2 + 1) // 2  # groups in the first of two gathers per chunk
    nregA = nc.gpsimd.to_reg(ngA * 128)
    nregB = nc.gpsimd.to_reg((G // 2 - ngA) * 128)

    with tile.TileContext(nc) as tc:
        with (
            nc.allow_low_precision(
                reason="bf16 staging; tolerance 2e-2, measured ~1e-2"),
            tc.tile_pool(name="const", bufs=1) as cpool,
            tc.tile_pool(name="meta", bufs=1) as mpool,
            tc.tile_pool(name="nproj", bufs=3) as npool,
            tc.tile_pool(name="edgea", bufs=4) as apool,
            tc.tile_pool(name="edgeb", bufs=4) as bpool,
            tc.tile_pool(name="small", bufs=4) as spool,
            tc.tile_pool(name="fin", bufs=2) as fpool,
            tc.tile_pool(name="ps_x", bufs=2, space="PSUM") as ps_x,
            tc.tile_pool(name="ps_n", bufs=2, space="PSUM") as ps_n,
        ):
            # load the GPSIMD library holding InstDMAGatherAnt (stays loaded;
            # no other Pool custom instructions are used)
            # ---- constants & resident metadata
            W_t = cpool.tile([IN_DIM, EMBED], BF16)
            nc.sync.dma_start(out=W_t[:], in_=W_e[:])
            Wb_t = cpool.tile([128, EMBED], F32)
            nc.sync.dma_start(out=Wb_t[:], in_=Wb_e[:])
            We_t = cpool.tile([EDGE_DIM + 1, EMBED], BF16)
            nc.sync.dma_start(out=We_t[:], in_=We_e[:])
            a_t = cpool.tile([128, EMBED], BF16)
            nc.sync.dma_start(out=a_t[:], in_=a_e[:])

            s16_t = mpool.tile([128, NTILE * G * 8], I16)
            nc.sync.dma_start(out=s16_t[:], in_=s16_e[:])
            rtab = mpool.tile([128, NTILE, EMBED], BF16)  # own-range projections

            for _ in range(reps):
                # ---- phase 1: node projection -> table (replicated)
                for t in range(NPAD // (128 * MR)):
                    nf_t = npool.tile([IN_DIM, 128 * MR], BF16, tag="nf")
                    nc.sync.dma_start(
                        out=nf_t[:],
                        in_=nfT[:, t * 128 * MR:(t + 1) * 128 * MR])
                    ps = ps_x.tile([128, SUB, EMBED], F32, space="PSUM", tag="px")
                    for g in range(MR):
                        nc.tensor.matmul(
                            out=ps[:, g, :],
                            lhsT=nf_t[:, g * 128:(g + 1) * 128],
                            rhs=W_t[:], start=True, stop=True)
                    nb = npool.tile([128, MR, EMBED], F32, tag="nb")
                    nc.vector.tensor_add(nb[:], ps[:, :MR, :], _ap3(Wb_t[:], MR))
                    # nfT columns are host-permuted so (group 2b, 2b+1) =
                    # (even, odd) nodes of 256-node block b; one partition
                    # then holds a full 512B pair row of the table
                    nc.sync.dma_start(
                        out=bass.AP(tap.tensor, t * 128 * MR * EMBED,
                                    [[2 * EMBED, 128],
                                     [128 * 2 * EMBED, MR // 2],
                                     [1, 2 * EMBED]]),
                        in_=nb[:].rearrange("p (c two) d -> p c (two d)",
                                            two=2))

                # ---- phase 1b: own-range projection -> SBUF rtab
                for t in range(NTILE // MR2):
                    nf_t = npool.tile([IN_DIM, 128 * MR2], BF16, tag="nf2")
                    nc.sync.dma_start(
                        out=nf_t[:],
                        in_=rnfT[:, t * 128 * MR2:(t + 1) * 128 * MR2])
                    ps = ps_x.tile([128, SUB, EMBED], F32, space="PSUM", tag="px")
                    for g in range(MR2):
                        nc.tensor.matmul(
                            out=ps[:, g, :],
                            lhsT=nf_t[:, g * 128:(g + 1) * 128],
                            rhs=W_t[:], start=True, stop=True)
                    nc.vector.tensor_add(
                        rtab[:, t * MR2:(t + 1) * MR2, :], ps[:, :MR2, :],
                        _ap3(Wb_t[:], MR2))

                # ---- phase 2: receiver tiles (2 parity chunks of SUB groups)
                for t in range(NTILE):
                    psn = ps_n.tile([128, PAY], F32, space="PSUM", tag="pn")
                    # host-precomputed one-hot R for the whole tile (fp8:
                    # 0/1 exact, so psn/psx matmuls are numerically exact)
                    R_t = spool.tile([128, G, 128], F8, tag="R")
                    nc.sync.dma_start(
                        out=R_t[:],
                        in_=Rh_e[:, t * G * 128:(t + 1) * G * 128].rearrange(
                            "p (g j) -> p g j", j=128))
                    RT_t = spool.tile([128, G, 128], F8, tag="RT")
                    nc.sync.dma_start(
                        out=RT_t[:],
                        in_=RhT_e[:, t * G * 128:(t + 1) * G * 128].rearrange(
                            "p (g j) -> p g j", j=128))
                    for hi in range(2):
                        ng = SUB
                        g0 = t * G + hi * SUB
                        c0 = t * C + hi * SUB * 128
                        eft_t = apool.tile([EDGE_DIM + 1, ng * 128], BF16,
                                           tag="eft")
                        nc.sync.dma_start(
                            out=eft_t[:], in_=eftT[:, c0:c0 + ng * 128])
                        # Two batched gathers per chunk (the Q7 dma_gather
                        # ucode caps out between 1024 and 1280 indices per
                        # instruction; SUB*128 = 1664). Different queues ->
                        # different Q7 cpu pairs gen descriptors concurrently.
                        g_t = apool.tile([128, ng, EMBED], F32, tag="gat")
                        view = ev_view if hi == 0 else od_view
                        for i, (lo, n, rg) in enumerate(
                                ((0, ngA, nregA), (ngA, ng - ngA, nregB))):
                            nc.gpsimd.dma_gather(
                                out_ap=g_t[:, lo:lo + n, :],
                                in_ap=view,
                                idxs_ap=s16_t[:, (g0 + lo) * 8:
                                              (g0 + lo + n) * 8],
                                num_idxs=n * 128, num_idxs_reg=rg,
                                elem_size=EMBED, elem_step=2 * EMBED,
                                queue_num=(2 * (2 * t + hi) + i) % NQ)
                        gb = hi * SUB  # group base within the tile's R
                        # x_psum = edge projection + recv-row expansion
                        psx = ps_x.tile([128, ng, EMBED], F32, space="PSUM",
                                        tag="px")
                        for g in range(ng):
                            nc.tensor.matmul(
                                out=psx[:, g, :],
                                lhsT=eft_t[:, g * 128:(g + 1) * 128],
                                rhs=We_t[:], start=True, stop=False)
                            nc.tensor.matmul(
                                out=psx[:, g, :],
                                lhsT=RT_t[:, gb + g, :],
                                rhs=rtab[:, t, :], start=False,
                                stop=True)
                        x_t = bpool.tile([128, ng, EMBED], xdt, tag="x")
                        xf = x_t[:].rearrange("p c d -> p (c d)")
                        nc.vector.tensor_add(
                            xf, g_t[:].rearrange("p c d -> p (c d)"),
                            psx[:].rearrange("p c d -> p (c d)"))
                        xm_t = bpool.tile([128, ng, EMBED], xdt, tag="xm")
                        xmf = xm_t[:].rearrange("p c d -> p (c d)")
                        # mish(x) = x*(1 - 2/w), w = (e^x+1)^2 + 1
                        # ACT: u=e^x, v=(u+1)^2, w=v+1; DVE: recip, t=-2r+1,
                        # xm=x*t
                        u_t = bpool.tile([128, ng * EMBED], xdt, tag="mu")
                        tb_t = bpool.tile([128, ng * EMBED], xdt, tag="mtb")
                        nc.scalar.activation(
                            u_t[:], xf, mybir.ActivationFunctionType.Exp)
                        nc.scalar.activation(
                            tb_t[:], u_t[:],
                            mybir.ActivationFunctionType.Square, bias=1.0)
                        nc.scalar.activation(
                            u_t[:], tb_t[:],
                            mybir.ActivationFunctionType.Identity, bias=1.0)
                        nc.vector.reciprocal(tb_t[:], u_t[:])
                        nc.vector.tensor_scalar(
                            u_t[:], tb_t[:], -2.0, 1.0,
                            mybir.AluOpType.mult, mybir.AluOpType.add)
                        nc.vector.tensor_mul(xmf, xf, u_t[:])
                        lg_t = bpool.tile([128, ng * EMBED], xdt, tag="lg")
                        nc.vector.tensor_mul(
                            lg_t[:].rearrange("p (c d) -> p c d", d=EMBED),
                            xm_t[:], _ap3(a_t[:], ng))
                        l_t = spool.tile([128, ng * HEADS], BF16, tag="l")

                        nc.vector.tensor_reduce(
                            l_t[:].rearrange("p (q o) -> p q o", o=1),
                            lg_t[:].rearrange("p (q i) -> p q i", i=8),
                            axis=mybir.AxisListType.X, op=mybir.AluOpType.add)
                        pay_t = bpool.tile([128, ng, PAY], BF16, tag="pay")
                        wv = pay_t[:, :, EMBED:]  # [128, ng, 8]
                        # no mask needed: padded slots carry rrel=255, so R
                        # has an all-zero column and psn ignores them
                        nc.scalar.activation(
                            wv, l_t[:].rearrange("p (c h) -> p c h", h=HEADS),
                            mybir.ActivationFunctionType.Exp)
                        if batch_payload:
                            out4 = pay_t[:, :, :EMBED].rearrange(
                                "p c (h o) -> p c h o", o=8)
                            src4 = g_t[:].rearrange("p c (h o) -> p c h o", o=8)
                            w4 = _inner_b(wv, 8)
                            nc.vector.tensor_mul(out4, src4, w4)
                        else:
                            for g in range(ng):
                                nc.vector.tensor_mul(
                                    pay_t[:, g, :EMBED].rearrange(
                                        "p (h o) -> p h o", o=8),
                                    g_t[:, g, :].rearrange(
                                        "p (h o) -> p h o", o=8),
                                    _inner_b(pay_t[:, g, EMBED:], 8))
                        for g in range(ng):
                            nc.tensor.matmul(
                                out=psn[:], lhsT=R_t[:, gb + g, :],
                                rhs=pay_t[:, g, :],
                                start=(hi == 0 and g == 0),
                                stop=(hi == 1 and g == ng - 1))
                    den = fpool.tile([128, HEADS], F32, tag="den")
                    nc.vector.tensor_scalar_add(den[:], psn[:, EMBED:], 1e-30)
                    rec = fpool.tile([128, HEADS], F32, tag="rec")
                    nc.vector.reciprocal(rec[:], den[:])
                    ot = fpool.tile([128, EMBED], F32, tag="ot")
                    nc.vector.tensor_mul(
                        ot[:].rearrange("p (h o) -> p h o", o=8),
                        psn[:, :EMBED].rearrange("p (h o) -> p h o", o=8),
                        _inner_b(rec[:], 8))
                    nc.sync.dma_start(
                        out=out_e[t * 128:(t + 1) * 128, :], in_=ot[:])

    _split_excess_waits(nc)
    mybir.codegen_inst_isa_subclasses(nc)
    return nc


def host_prep(node_features, edge_features, senders, receivers,
              W_kernel, W_bias, We_kernel, We_bias, a):
    """Sort edges by (receiver tile, sender parity); build per-core
    tile-padded streams. Even-sender edges fill the first G/2 groups of each
    receiver tile, odd-sender the last G/2."""
    node_features = np.asarray(node_features, np.float32)
    edge_features = np.asarray(edge_features, np.float32)
    senders = np.asarray(senders, np.int64)
    receivers = np.asarray(receivers, np.int64)

    gtile = receivers // 128
    par = senders & 1
    order = np.lexsort((par, gtile))
    s_s = senders[order]
    ef_s = edge_features[order]
    r_s = receivers[order]
    seg = gtile[order] * 2 + par[order]  # (tile, parity) segment id

    n_gtiles = NPAD // 128  # 392
    counts = np.bincount(seg, minlength=n_gtiles * 2)
    SUB = max(MR2, int(np.ceil(counts.max() / 128)))
    G = 2 * SUB
    C = G * 128

    starts = np.zeros(n_gtiles * 2, np.int64)
    starts[1:] = np.cumsum(counts)[:-1]
    within = np.arange(len(seg)) - starts[seg]
    slot = seg * SUB * 128 + within

    tot_slots = n_gtiles * C
    s_pad = np.zeros(tot_slots, np.int64)
    # padded slots keep an all-zero R row -> the psn segment-sum matmul
    # drops them (no separate mask needed)
    ef_pad = np.zeros((tot_slots, EDGE_DIM + 1), np.float32)
    s_pad[slot] = s_s
    ef_pad[slot, :EDGE_DIM] = ef_s
    ef_pad[:, EDGE_DIM] = 1.0  # ones-row for folded We_bias

    # int16 gather indices: s>>1, wrapped in 16 partitions per chunk-segment,
    # replicated across the 8 Q7 core stripes
    s16 = (s_pad >> 1).astype(np.uint16)
    wi = s16.reshape(n_gtiles * 2, SUB * 8, 16).transpose(2, 0, 1)
    wi = wi.reshape(16, n_gtiles * G * 8)
    s16_full = np.tile(wi, (8, 1)).view(np.int16)  # [128, n_gtiles*G*8]

    nf_pad = np.zeros((NPAD, IN_DIM), np.float32)
    nf_pad[:N_NODES] = node_features
    # permute so each 256-node block is [evens then odds] (paired table write)
    pi = np.arange(NPAD).reshape(-1, 128, 2)  # [block, pair, parity]
    pi = np.concatenate([pi[:, :, 0], pi[:, :, 1]], axis=1).reshape(-1)
    nfT = np.ascontiguousarray(nf_pad[pi].T)

    Wb_rep = np.tile(np.asarray(W_bias, np.float32)[None, :], (128, 1))
    We_ext = np.concatenate(
        [np.asarray(We_kernel, np.float32),
         np.asarray(We_bias, np.float32)[None, :]], axis=0)
    a64 = np.tile(np.asarray(a, np.float32).reshape(-1)[None, :], (128, 1))

    # host-precomputed one-hot R in fp8 (0/1 exact): [slot, j] -> kernel
    # layout [128p, NTILE*G*128] with col (t*G+g)*128+j <- slot t*C+g*128+p
    R_pad = np.zeros((tot_slots, 128), ml_dtypes.float8_e4m3)
    R_pad[slot, (r_s % 128).astype(np.int64)] = 1.0

    def wrap_idx(arr):
        # [49*C] -> [128, NTILE*G]: col t*G+g, partition p <- slot t*C+g*128+p
        return np.ascontiguousarray(
            arr.reshape(NTILE, G, 128).transpose(2, 0, 1).reshape(128, NTILE * G))

    in_maps = []
    for c in range(N_CORES):
        lo, hi = c * NTILE * C, (c + 1) * NTILE * C
        in_maps.append({
            "nfT": nfT.astype(ml_dtypes.bfloat16),
            "rnfT": np.ascontiguousarray(
                nf_pad[c * NPC:(c + 1) * NPC].T).astype(ml_dtypes.bfloat16),
            "eftT": np.ascontiguousarray(
                ef_pad[lo:hi].T.astype(ml_dtypes.bfloat16)),
            "s16": np.ascontiguousarray(
                s16_full[:, c * NTILE * G * 8:(c + 1) * NTILE * G * 8]),
            "Rh": np.ascontiguousarray(
                R_pad[lo:hi].reshape(NTILE, G, 128, 128).transpose(
                    2, 0, 1, 3).reshape(128, NTILE * G * 128)),
            "RhT": np.ascontiguousarray(
                R_pad[lo:hi].reshape(NTILE, G, 128, 128).transpose(
                    3, 0, 1, 2).reshape(128, NTILE * G * 128)),
            "W": np.asarray(W_kernel, np.float32).astype(ml_dtypes.bfloat16),
            "Wb": Wb_rep,
            "We_ext": We_ext.astype(ml_dtypes.bfloat16),
            "a64": a64.astype(ml_dtypes.bfloat16),
        })
    return in_maps, G


def _build_runner(nc, n_cores):
    import time
    import jax
    from jax.sharding import Mesh, PartitionSpec
    from jax.experimental.shard_map import shard_map
    from concourse import bass2jax
    from concourse.bass2jax import _bass_exec_p, install_neuronx_cc_hook

    install_neuronx_cc_hook()
    partition_name = nc.partition_id_tensor.name if nc.partition_id_tensor else None
    in_names, out_names, out_avals, zero_outs = [], [], [], []
    for alloc in nc.m.functions[0].allocations:
        if not isinstance(alloc, mybir.MemoryLocationSet):
            continue
        name = alloc.memorylocations[0].name
        if alloc.kind == "ExternalInput":
            if name != partition_name:
                in_names.append(name)
        elif alloc.kind == "ExternalOutput":
            out_names.append(name)
            shape = tuple(alloc.tensor_shape)
            dtype = mybir.dt.np(alloc.dtype)
            out_avals.append(jax.core.ShapedArray(shape, dtype))
            zero_outs.append(np.zeros(shape, dtype))
    n_params = len(in_names)
    n_outs = len(out_avals)
    all_in_names = list(in_names) + list(out_names)
    if partition_name is not None:
        all_in_names.append(partition_name)

    def _body(*args):
        operands = list(args)
        if partition_name is not None:
            operands.append(bass2jax.partition_id_tensor())
        return tuple(_bass_exec_p.bind(
            *operands,
            out_avals=tuple(out_avals),
            in_names=tuple(all_in_names),
            out_names=tuple(out_names),
            lowering_input_output_aliases=(),
            sim_require_finite=True,
            sim_require_nnan=True,
            nc=nc,
        ))

    donate = tuple(range(n_params, n_params + n_outs))
    devices = jax.devices()[:n_cores]
    mesh = Mesh(np.asarray(devices), ("core",))
    in_specs = (PartitionSpec("core"),) * (n_params + n_outs)
    out_specs = (PartitionSpec("core"),) * len(out_names)
    jfn = jax.jit(
        shard_map(_body, mesh=mesh, in_specs=in_specs, out_specs=out_specs,
                  check_rep=False),
        donate_argnums=donate, keep_unused=True)

    def fn(in_maps):
        concat_in = [
            np.concatenate([np.asarray(in_maps[c][n]) for c in range(n_cores)], 0)
            for n in in_names
        ]
        concat_zeros = [np.zeros((n_cores * z.shape[0], *z.shape[1:]), z.dtype)
                        for z in zero_outs]
        t0 = time.perf_counter()
        out_arrs = jfn(*concat_in, *concat_zeros)
        out_arrs = [np.asarray(o) for o in out_arrs]
        dt = time.perf_counter() - t0
        return [
            {n: out_arrs[i].reshape(n_cores, *out_avals[i].shape)[c]
             for i, n in enumerate(out_names)}
            for c in range(n_cores)
        ], dt

    return fn


_CACHE = {}


def kernel(node_features, edge_features, global_features, senders, receivers,
           W_kernel, W_bias, We_kernel, We_bias, a):
    in_maps, G = host_prep(node_features, edge_features, senders, receivers,
                           W_kernel, W_bias, We_kernel, We_bias, a)
    if _CACHE.get("G") != G:
        nc = build_nc(G)
        _CACHE["fn"] = _build_runner(nc, N_CORES)
        _CACHE["G"] = G
    res, dt = _CACHE["fn"](in_maps)
    _CACHE["last_dt"] = dt
    full = np.concatenate([r["out_shard"] for r in res], axis=0)
    return full[:N_NODES].astype(np.float32)
